# revision 17
# baseline (speedup 1.0000x reference)
"""Trainium2 Bass kernel for DetectionLayer (refine + per-class NMS).

Contract: kernel(rois, probs, deltas) with FULL inputs
  rois   [16, 4096, 4]   f32
  probs  [16, 4096, 81]  f32
  deltas [16, 4096, 81, 4] f32
returns [16, 100, 6] f32 detections, matching the jax reference.

Sharding: pure data parallel - 2 images per core across 8 NeuronCores.

Fast path (always): DMA both images' probs, count elements >= 0.7 with a
DVE is_ge+accum / ACT sign+accum split, sum via PE ones-matmul.  The
zeroed output is DMA'd to HBM up front.
Guard (tc.If, only when count > 0): deltas load, per-argmax-class box
refine, and a fixed 100-iteration per-class NMS per image, then the real
detections overwrite the zeros in HBM.
"""

import os as _os

import numpy as np

import concourse.bacc as bacc
import concourse.bass as bass
import concourse.bass_isa as bass_isa
import concourse.mybir as mybir
from concourse.tile import TileContext

B = 16              # full batch
NCORES = 8
BPC = B // NCORES   # images per core
N = 4096            # rois per image
C = 81              # classes
K = 100             # detection_max_instances
P = 128             # SBUF partitions
NP = N // P         # rois per partition per image (32)
NEG = -1e9
MIN_CONF = 0.7
NMS_T = 0.3
F32 = mybir.dt.float32
I32 = mybir.dt.int32

# gate split: DVE handles rois [0, NA), ACT handles [NA, NP) of each image
NA = int(_os.environ.get("DETK_NA", "18"))   # img0 DVE rois
NH = NP // 2                                 # img1 DMA half split
N1A = int(_os.environ.get("DETK_N1A", "7"))  # img1-half1 DVE rois
DEBUG = _os.environ.get("DETK_DEBUG", "0") == "1"
NOGUARD = _os.environ.get("DETK_NOGUARD", "0") == "1"
NB = max(NP - NA, NP - NH - N1A)             # ACT scratch rois (max chunk)
# total elements processed by ACT chunks (sign-sum offset)
TOTAL_B_ELEMS = float(P * C * ((NP - NA) + (NP - NH - N1A)))


def _refine_image(nc, tc, sm, img, ptw, scw, rt, dt_, crev, state):
    """Cold path per image: select argmax-class delta, refine boxes, build
    NMS state. All tiles are [..] slices of twin tensors at free index img."""
    pt = ptw[:, img]          # [P, NP, C]
    scores = scw[:, img]      # [P, NP]

    nc.vector.reduce_max(scores, pt, axis=mybir.AxisListType.X)
    ge = sm.tile([P, NP], F32, tag=f"ge{img}")
    nc.vector.tensor_single_scalar(ge, scores, MIN_CONF,
                                   op=mybir.AluOpType.is_ge)

    # one-hot mask of argmax class: M = (probs == score), in place over probs
    m = pt
    nc.vector.tensor_tensor(
        m, pt, scores.unsqueeze(2).to_broadcast([P, NP, C]),
        op=mybir.AluOpType.is_equal,
    )

    # select argmax-class delta: deltas *= M (bcast over k), sum over c
    d_perm = dt_.rearrange("p n c k -> p n k c")
    nc.vector.tensor_tensor(
        d_perm, d_perm, m.unsqueeze(2).to_broadcast([P, NP, 4, C]),
        op=mybir.AluOpType.mult,
    )
    dsel = sm.tile([P, NP, 4], F32, tag=f"dsel{img}")
    nc.vector.reduce_sum(dsel, d_perm, axis=mybir.AxisListType.X)

    # class id = 80 - max((80-c) * M)  (ties -> smallest c, like argmax)
    nc.vector.tensor_tensor(m, m, crev, op=mybir.AluOpType.mult)
    cid = sm.tile([P, NP], F32, tag=f"cid{img}")
    nc.vector.reduce_max(cid, m, axis=mybir.AxisListType.X)
    nc.vector.tensor_scalar(
        out=cid, in0=cid, scalar1=-1.0, scalar2=float(C - 1),
        op0=mybir.AluOpType.mult, op1=mybir.AluOpType.add,
    )

    # bbox_std scaling (match reference op order exactly)
    nc.vector.tensor_scalar_mul(dsel[:, :, 0:2], dsel[:, :, 0:2], 0.1)
    nc.vector.tensor_scalar_mul(dsel[:, :, 2:4], dsel[:, :, 2:4], 0.2)

    # ---- apply deltas + clip (mirrors _apply_deltas fp32 op order) ----
    h = sm.tile([P, NP], F32, tag=f"h{img}")
    w = sm.tile([P, NP], F32, tag=f"w{img}")
    nc.vector.tensor_sub(h, rt[:, :, 2], rt[:, :, 0])
    nc.vector.tensor_sub(w, rt[:, :, 3], rt[:, :, 1])
    t1 = sm.tile([P, NP], F32, tag=f"t1{img}")
    t2 = sm.tile([P, NP], F32, tag=f"t2{img}")
    cy = sm.tile([P, NP], F32, tag=f"cy{img}")
    cx = sm.tile([P, NP], F32, tag=f"cx{img}")
    nc.vector.tensor_scalar_mul(t1, h, 0.5)
    nc.vector.tensor_add(t2, rt[:, :, 0], t1)
    nc.vector.tensor_mul(t1, dsel[:, :, 0], h)
    nc.vector.tensor_add(cy, t2, t1)
    nc.vector.tensor_scalar_mul(t1, w, 0.5)
    nc.vector.tensor_add(t2, rt[:, :, 1], t1)
    nc.vector.tensor_mul(t1, dsel[:, :, 1], w)
    nc.vector.tensor_add(cx, t2, t1)
    e = sm.tile([P, NP], F32, tag=f"e{img}")
    nc.scalar.activation(e, dsel[:, :, 2], mybir.ActivationFunctionType.Exp)
    nc.vector.tensor_mul(h, h, e)
    nc.scalar.activation(e, dsel[:, :, 3], mybir.ActivationFunctionType.Exp)
    nc.vector.tensor_mul(w, w, e)

    ref = sm.tile([P, NP, 4], F32, tag=f"ref{img}")
    nc.vector.tensor_scalar_mul(t1, h, 0.5)
    nc.vector.tensor_sub(ref[:, :, 0], cy, t1)
    nc.vector.tensor_add(ref[:, :, 2], cy, t1)
    nc.vector.tensor_scalar_mul(t2, w, 0.5)
    nc.vector.tensor_sub(ref[:, :, 1], cx, t2)
    nc.vector.tensor_add(ref[:, :, 3], cx, t2)
    nc.vector.tensor_scalar(
        out=ref, in0=ref, scalar1=0.0, scalar2=1.0,
        op0=mybir.AluOpType.max, op1=mybir.AluOpType.min,
    )

    # ---- NMS state ----
    sc = state["sc"][:, img]
    ob = state["ob"][:, img]
    ar = state["ar"][:, img]
    cat = state["cat"][:, img]
    negs = state["negs"]

    vf = sm.tile([P, NP], F32, tag=f"vf{img}")
    nc.vector.tensor_single_scalar(vf, cid, 0.5, op=mybir.AluOpType.is_ge)
    v = sm.tile([P, NP], mybir.dt.uint8, tag=f"v{img}")
    nc.vector.tensor_mul(v, vf, ge)
    nc.vector.tensor_copy(sc, negs)
    nc.vector.copy_predicated(sc, v, scores)

    nc.vector.scalar_tensor_tensor(
        out=ob, in0=cid.unsqueeze(2).to_broadcast([P, NP, 4]), scalar=2.0,
        in1=ref, op0=mybir.AluOpType.mult, op1=mybir.AluOpType.add,
    )
    ar2 = sm.tile([P, NP, 2], F32, tag=f"ar2{img}")
    nc.vector.tensor_sub(ar2, ob[:, :, 2:4], ob[:, :, 0:2])
    nc.vector.tensor_mul(ar, ar2[:, :, 0], ar2[:, :, 1])
    nc.vector.tensor_copy(cat[:, :, 0:4], ref)
    nc.vector.tensor_copy(cat[:, :, 4], cid)
    nc.vector.tensor_copy(cat[:, :, 5], scores)


def _nms_image(nc, tc, sm, img, det, state):
    """Cold path per image: fixed K-iteration NMS; rows past exhaustion are
    written as exact zeros (gm == NEG gate)."""
    sc = state["sc"][:, img]
    ob = state["ob"][:, img]
    ar = state["ar"][:, img]
    cat = state["cat"][:, img]
    negs = state["negs"]
    mr = state["mr"]

    with tc.For_i(0, K, name=f"nms{img}") as i:
        pm = sm.tile([P, 1], F32, tag=f"pm{img}")
        nc.vector.reduce_max(pm, sc, axis=mybir.AxisListType.X)
        gm = sm.tile([P, 1], F32, tag=f"gm{img}")
        nc.gpsimd.partition_all_reduce(gm, pm, channels=P,
                                       reduce_op=bass_isa.ReduceOp.max)
        msk = sm.tile([P, NP], F32, tag=f"msk{img}")
        nc.vector.tensor_tensor(msk, sc, gm.to_broadcast([P, NP]),
                                op=mybir.AluOpType.is_equal)
        mb6 = sm.tile([P, NP, 6], F32, tag=f"mb6{img}")
        nc.vector.tensor_tensor(
            mb6, cat, msk.unsqueeze(2).to_broadcast([P, NP, 6]),
            op=mybir.AluOpType.mult,
        )
        r6p = sm.tile([P, 6], F32, tag=f"r6p{img}")
        nc.vector.reduce_sum(r6p, mb6.rearrange("p n k -> p k n"),
                             axis=mybir.AxisListType.X)
        r6 = sm.tile([P, 6], F32, tag=f"r6{img}")
        nc.gpsimd.partition_all_reduce(r6, r6p, channels=P,
                                       reduce_op=bass_isa.ReduceOp.add)
        okm = sm.tile([P, 1], F32, tag=f"okm{img}")
        nc.vector.tensor_single_scalar(okm, gm, NEG * 0.5,
                                       op=mybir.AluOpType.is_gt)
        nc.vector.tensor_mul(r6, r6, okm.to_broadcast([P, 6]))
        nc.vector.tensor_copy(det[img][0:1, bass.ds(i * 6, 6)],
                              r6[0:1, :])

        sb = sm.tile([P, 4], F32, tag=f"sb{img}")
        nc.vector.scalar_tensor_tensor(
            out=sb, in0=r6[:, 4:5].to_broadcast([P, 4]), scalar=2.0,
            in1=r6[:, 0:4], op0=mybir.AluOpType.mult, op1=mybir.AluOpType.add,
        )
        mx = sm.tile([P, NP, 2], F32, tag=f"mx{img}")
        nc.vector.tensor_tensor(
            mx, ob[:, :, 0:2], sb[:, 0:2].unsqueeze(1).to_broadcast([P, NP, 2]),
            op=mybir.AluOpType.max,
        )
        mn = sm.tile([P, NP, 2], F32, tag=f"mn{img}")
        nc.vector.tensor_tensor(
            mn, ob[:, :, 2:4], sb[:, 2:4].unsqueeze(1).to_broadcast([P, NP, 2]),
            op=mybir.AluOpType.min,
        )
        nc.vector.tensor_sub(mn, mn, mx)
        nc.vector.tensor_scalar_max(mn, mn, 0.0)
        inter = sm.tile([P, NP], F32, tag=f"inter{img}")
        nc.vector.tensor_mul(inter, mn[:, :, 0], mn[:, :, 1])
        aa2 = sm.tile([P, 2], F32, tag=f"aa2{img}")
        nc.vector.tensor_sub(aa2, sb[:, 2:4], sb[:, 0:2])
        aa = sm.tile([P, 1], F32, tag=f"aa{img}")
        nc.vector.tensor_mul(aa, aa2[:, 0:1], aa2[:, 1:2])
        u = sm.tile([P, NP], F32, tag=f"u{img}")
        nc.vector.scalar_tensor_tensor(
            out=u, in0=ar, scalar=aa[:, 0:1], in1=inter,
            op0=mybir.AluOpType.add, op1=mybir.AluOpType.subtract,
        )
        sup = sm.tile([P, NP], mybir.dt.uint8, tag=f"sup{img}")
        nc.vector.scalar_tensor_tensor(
            out=sup, in0=u, scalar=NMS_T, in1=inter,
            op0=mybir.AluOpType.mult, op1=mybir.AluOpType.is_lt,
        )
        nc.vector.copy_predicated(sc, sup, negs)
        nc.vector.tensor_copy(mr[:, 0:1], gm)
        nc.vector.match_replace(out=sc, in_to_replace=mr, in_values=sc,
                                imm_value=NEG)


def build_nc():
    nc = bacc.Bacc("TRN2", target_bir_lowering=False)
    rois_t = nc.dram_tensor("rois", [BPC, N, 4], F32, kind="ExternalInput")
    probs_t = nc.dram_tensor("probs", [BPC, N, C], F32, kind="ExternalInput")
    deltas_t = nc.dram_tensor("deltas", [BPC, N, C, 4], F32, kind="ExternalInput")
    out_t = nc.dram_tensor("out", [BPC, K, 6], F32, kind="ExternalOutput")
    dbg_t = None
    if DEBUG:
        dbg_t = nc.dram_tensor("dbg", [1, 16], F32, kind="ExternalOutput")

    with TileContext(nc) as tc:
        with (
            tc.tile_pool(name="big", bufs=1) as big,
            tc.tile_pool(name="small", bufs=1) as sm,
            tc.tile_pool(name="psum", bufs=1, space="PSUM") as pp,
        ):
            # ---------------- fast path ----------------
            # probs for both images in one twin tile; one DMA per image,
            # issued on separate HWDGE queues (SP + Act) so descriptor
            # generation for the two transfers runs in parallel
            ptw = big.tile([P, BPC, NP, C], F32, tag="probs")
            p1 = probs_t[1].rearrange("(p n) c -> p n c", p=P)
            nc.scalar.dma_start(out=ptw[:, 1, 0:NH], in_=p1[:, 0:NH])
            nc.sync.dma_start(
                out=ptw[:, 0],
                in_=probs_t[0].rearrange("(p n) c -> p n c", p=P))
            nc.scalar.dma_start(out=ptw[:, 1, NH:NP], in_=p1[:, NH:NP])

            det0 = sm.tile([1, K * 6], F32, tag="det0")
            det1 = sm.tile([1, K * 6], F32, tag="det1")
            det = [det0, det1]
            nc.vector.memset(det0, 0.0)
            nc.gpsimd.memset(det1, 0.0)

            # zeros out-DMA up front; real detections overwrite in the guard
            out_aps = []
            for img in range(BPC):
                ap = out_t[img].rearrange("k s -> (k s)").unsqueeze(0)
                out_aps.append(ap)
                nc.sync.dma_start(out=ap, in_=det[img][0:1])

            # element count >= MIN_CONF: DVE is_ge+sum chunks (coeff 2)
            # and ACT sign+sum chunks (coeff 1), 5 accum columns total
            cnt = sm.tile([P, 6], F32, tag="cnt")
            nc.vector.memset(cnt, 0.0)
            scrA = sm.tile([P, NA, C], mybir.dt.uint8, tag="scrA")
            scrB = sm.tile([P, NB, C], mybir.dt.bfloat16, tag="scrB")
            biasT = sm.tile([P, 1], F32, tag="biasT")
            nc.gpsimd.memset(biasT, -MIN_CONF)

            def dve_count(src_ap, col, nr):
                nc.vector.tensor_scalar(
                    out=scrA[:, 0:nr], in0=src_ap, scalar1=MIN_CONF,
                    scalar2=None, op0=mybir.AluOpType.is_ge,
                    op1=mybir.AluOpType.add,
                    accum_out=cnt[:, col:col + 1],
                )

            def act_count(src_ap, col, nr):
                nc.scalar.activation(
                    scrB[:, 0:nr], src_ap,
                    mybir.ActivationFunctionType.Sign, bias=biasT[:, 0:1],
                    accum_out=cnt[:, col:col + 1],
                )

            # img1 half0: DVE whole half (arrives first)
            dve_count(ptw[:, 1, 0:NH], 0, NH)
            # img0 (one start): DVE [0:NA], ACT [NA:NP]
            dve_count(ptw[:, 0, 0:NA], 1, NA)
            act_count(ptw[:, 0, NA:NP], 3, NP - NA)
            # img1 half1 (last to land): DVE + ACT split
            dve_count(ptw[:, 1, NH:NH + N1A], 2, N1A)
            act_count(ptw[:, 1, NH + N1A:NP], 4, NP - NH - N1A)

            ones = sm.tile([P, 1], F32, tag="ones")
            nc.vector.memset(ones, 1.0)
            # g = 2*sum(DVE counts) + sum(ACT sign sums) + #ACT-elems
            #   = 2 * (total elements >= MIN_CONF)   (exact in f32)
            nc.vector.tensor_scalar_mul(cnt[:, 0:3], cnt[:, 0:3], 2.0)
            csum = pp.tile([1, 6], F32, tag="csum")
            nc.tensor.matmul(csum, ones, cnt, start=True, stop=True)
            ga = sm.tile([1, 1], F32, tag="ga")
            nc.vector.reduce_sum(ga, csum, axis=mybir.AxisListType.X)
            gi = sm.tile([1, 1], I32, tag="gi")
            nc.vector.tensor_scalar(
                out=gi, in0=ga, scalar1=TOTAL_B_ELEMS, scalar2=None,
                op0=mybir.AluOpType.add, op1=mybir.AluOpType.bypass,
            )

            if DEBUG:
                dbgs = sm.tile([1, 8], F32, tag="dbgs")
                nc.vector.memset(dbgs, 0.0)
                nc.vector.tensor_copy(dbgs[0:1, 0:4], cs)
                nc.vector.tensor_copy(dbgs[0:1, 4:5], ga)
                nc.vector.tensor_copy(dbgs[0:1, 5:6], gb)
                nc.sync.dma_start(out=dbg_t[0:1, 0:8], in_=dbgs)

            gv = nc.values_load(gi[0:1, 0:1], min_val=0,
                                max_val=2 * BPC * N * C,
                                skip_runtime_bounds_check=True)

            # ---------------- guarded cold path ----------------
            if not NOGUARD:
              with tc.If(gv >= 1):
                crev = sm.tile([P, NP, C], F32, tag="crev")
                nc.gpsimd.iota(crev, pattern=[[0, NP], [-1, C]], base=C - 1,
                               channel_multiplier=0,
                               allow_small_or_imprecise_dtypes=True)
                negs = sm.tile([P, NP], F32, tag="negs")
                nc.gpsimd.memset(negs, NEG)
                mr = sm.tile([P, 8], F32, tag="mr")
                nc.gpsimd.memset(mr, NEG)

                sc_w = sm.tile([P, BPC, NP], F32, tag="sc")
                ob_w = sm.tile([P, BPC, NP, 4], F32, tag="ob")
                ar_w = sm.tile([P, BPC, NP], F32, tag="ar")
                cat_w = sm.tile([P, BPC, NP, 6], F32, tag="cat")
                state = {
                    "negs": negs,
                    "mr": mr,
                    "sc": sc_w,
                    "ob": ob_w,
                    "ar": ar_w,
                    "cat": cat_w,
                }
                scw = sm.tile([P, BPC, NP], F32, tag="scores")

                for img in range(BPC):
                    rt = sm.tile([P, NP, 4], F32, tag=f"rois{img}")
                    nc.sync.dma_start(
                        out=rt,
                        in_=rois_t[img].rearrange("(p n) k -> p n k", p=P))
                    dt_ = big.tile([P, NP, C, 4], F32, tag=f"deltas{img}")
                    dsrc = deltas_t[img].rearrange("(p n) c k -> p n c k", p=P)
                    for s in range(8):
                        sl = slice(16 * s, 16 * s + 16)
                        nc.sync.dma_start(out=dt_[sl], in_=dsrc[sl])
                    _refine_image(nc, tc, sm, img, ptw, scw, rt, dt_, crev,
                                  state)
                if DEBUG:
                    pmd = sm.tile([P, 4], F32, tag="pmd")
                    nc.vector.reduce_max(pmd[:, 0:1], state["sc"][:, 0],
                                         axis=mybir.AxisListType.X)
                    nc.vector.reduce_max(pmd[:, 1:2], state["sc"][:, 1],
                                         axis=mybir.AxisListType.X)
                    nc.vector.reduce_max(pmd[:, 2:3], scw[:, 0],
                                         axis=mybir.AxisListType.X)
                    nc.vector.reduce_max(pmd[:, 3:4], scw[:, 1],
                                         axis=mybir.AxisListType.X)
                    pmg = sm.tile([P, 4], F32, tag="pmg")
                    nc.gpsimd.partition_all_reduce(
                        pmg, pmd, channels=P, reduce_op=bass_isa.ReduceOp.max)
                    nc.sync.dma_start(out=dbg_t[0:1, 8:12], in_=pmg[0:1, :])
                for img in range(BPC):
                    _nms_image(nc, tc, sm, img, det, state)
                if DEBUG:
                    dbgs2 = sm.tile([1, 4], F32, tag="dbgs2")
                    nc.vector.tensor_copy(dbgs2[0:1, 0:2], det[0][0:1, 0:2])
                    nc.vector.tensor_copy(dbgs2[0:1, 2:4], det[1][0:1, 0:2])
                    nc.sync.dma_start(out=dbg_t[0:1, 12:16], in_=dbgs2)
                for img in range(BPC):
                    fap = out_t[img].rearrange("k s -> (k s)").unsqueeze(0)
                    nc.sync.dma_start(out=fap, in_=det[img][0:1])
    nc.compile()
    return nc


LAST_RESULTS = None  # BassKernelResults of the most recent kernel() call


def kernel(rois, probs, deltas):
    global LAST_RESULTS
    from concourse import bass_utils

    nc = build_nc()
    in_maps = []
    for c in range(NCORES):
        sl = slice(c * BPC, (c + 1) * BPC)
        in_maps.append({
            "rois": np.ascontiguousarray(rois[sl], dtype=np.float32),
            "probs": np.ascontiguousarray(probs[sl], dtype=np.float32),
            "deltas": np.ascontiguousarray(deltas[sl], dtype=np.float32),
        })
    res = bass_utils.run_bass_kernel_spmd(nc, in_maps, core_ids=list(range(NCORES)))
    LAST_RESULTS = res
    return np.concatenate([r["out"] for r in res.results], axis=0)


if __name__ == "__main__":
    rng = np.random.default_rng(0)
    out = kernel(
        rng.random((B, N, 4), np.float32),
        rng.random((B, N, C), np.float32),
        rng.standard_normal((B, N, C, 4)).astype(np.float32),
    )
    print(out.shape, np.abs(out).max())


# revision 19
# speedup vs baseline: 1.0152x; 1.0152x over previous
"""Trainium2 Bass kernel for DetectionLayer (refine + per-class NMS).

Contract: kernel(rois, probs, deltas) with FULL inputs
  rois   [16, 4096, 4]   f32
  probs  [16, 4096, 81]  f32
  deltas [16, 4096, 81, 4] f32
returns [16, 100, 6] f32 detections, matching the jax reference.

Sharding: pure data parallel - 2 images per core across 8 NeuronCores.

Fast path (always): DMA both images' probs, count elements >= 0.7 with a
DVE is_ge+accum / ACT sign+accum split, sum via PE ones-matmul.  The
zeroed output is DMA'd to HBM up front.
Guard (tc.If, only when count > 0): deltas load, per-argmax-class box
refine, and a fixed 100-iteration per-class NMS per image, then the real
detections overwrite the zeros in HBM.
"""

import os as _os

import numpy as np

import concourse.bacc as bacc
import concourse.bass as bass
import concourse.bass_isa as bass_isa
import concourse.mybir as mybir
from concourse.tile import TileContext

B = 16              # full batch
NCORES = 8
BPC = B // NCORES   # images per core
N = 4096            # rois per image
C = 81              # classes
K = 100             # detection_max_instances
P = 128             # SBUF partitions
NP = N // P         # rois per partition per image (32)
NEG = -1e9
MIN_CONF = 0.7
NMS_T = 0.3
F32 = mybir.dt.float32
I32 = mybir.dt.int32

# gate split: DVE handles rois [0, NA), ACT handles [NA, NP) of each image
DEBUG = _os.environ.get("DETK_DEBUG", "0") == "1"
NOGUARD = _os.environ.get("DETK_NOGUARD", "0") == "1"
# gate work split across engines, per DMA chunk (A=img0, B=img1[0:BS], C=rest)
BS = int(_os.environ.get("DETK_BS", "22"))    # img1 sync-queue rois
A_DVE = int(_os.environ.get("DETK_ADVE", "18"))
B_DVE = int(_os.environ.get("DETK_BDVE", "17"))
A_ACT = NP - A_DVE
B_ACT = BS - B_DVE
C_ACT = NP - BS
# total elements processed by ACT (sign-sum) chunks
TOTAL_B_ELEMS = float(P * C * (A_ACT + B_ACT + C_ACT))


def _refine_image(nc, tc, sm, img, ptw, scw, rt, dt_, crev, state):
    """Cold path per image: select argmax-class delta, refine boxes, build
    NMS state. All tiles are [..] slices of twin tensors at free index img."""
    pt = ptw[:, img]          # [P, NP, C]
    scores = scw[:, img]      # [P, NP]

    nc.vector.reduce_max(scores, pt, axis=mybir.AxisListType.X)
    ge = sm.tile([P, NP], F32, tag=f"ge{img}")
    nc.vector.tensor_single_scalar(ge, scores, MIN_CONF,
                                   op=mybir.AluOpType.is_ge)

    # one-hot mask of argmax class: M = (probs == score), in place over probs
    m = pt
    nc.vector.tensor_tensor(
        m, pt, scores.unsqueeze(2).to_broadcast([P, NP, C]),
        op=mybir.AluOpType.is_equal,
    )

    # select argmax-class delta: deltas *= M (bcast over k), sum over c
    d_perm = dt_.rearrange("p n c k -> p n k c")
    nc.vector.tensor_tensor(
        d_perm, d_perm, m.unsqueeze(2).to_broadcast([P, NP, 4, C]),
        op=mybir.AluOpType.mult,
    )
    dsel = sm.tile([P, NP, 4], F32, tag=f"dsel{img}")
    nc.vector.reduce_sum(dsel, d_perm, axis=mybir.AxisListType.X)

    # class id = 80 - max((80-c) * M)  (ties -> smallest c, like argmax)
    nc.vector.tensor_tensor(m, m, crev, op=mybir.AluOpType.mult)
    cid = sm.tile([P, NP], F32, tag=f"cid{img}")
    nc.vector.reduce_max(cid, m, axis=mybir.AxisListType.X)
    nc.vector.tensor_scalar(
        out=cid, in0=cid, scalar1=-1.0, scalar2=float(C - 1),
        op0=mybir.AluOpType.mult, op1=mybir.AluOpType.add,
    )

    # bbox_std scaling (match reference op order exactly)
    nc.vector.tensor_scalar_mul(dsel[:, :, 0:2], dsel[:, :, 0:2], 0.1)
    nc.vector.tensor_scalar_mul(dsel[:, :, 2:4], dsel[:, :, 2:4], 0.2)

    # ---- apply deltas + clip (mirrors _apply_deltas fp32 op order) ----
    h = sm.tile([P, NP], F32, tag=f"h{img}")
    w = sm.tile([P, NP], F32, tag=f"w{img}")
    nc.vector.tensor_sub(h, rt[:, :, 2], rt[:, :, 0])
    nc.vector.tensor_sub(w, rt[:, :, 3], rt[:, :, 1])
    t1 = sm.tile([P, NP], F32, tag=f"t1{img}")
    t2 = sm.tile([P, NP], F32, tag=f"t2{img}")
    cy = sm.tile([P, NP], F32, tag=f"cy{img}")
    cx = sm.tile([P, NP], F32, tag=f"cx{img}")
    nc.vector.tensor_scalar_mul(t1, h, 0.5)
    nc.vector.tensor_add(t2, rt[:, :, 0], t1)
    nc.vector.tensor_mul(t1, dsel[:, :, 0], h)
    nc.vector.tensor_add(cy, t2, t1)
    nc.vector.tensor_scalar_mul(t1, w, 0.5)
    nc.vector.tensor_add(t2, rt[:, :, 1], t1)
    nc.vector.tensor_mul(t1, dsel[:, :, 1], w)
    nc.vector.tensor_add(cx, t2, t1)
    e = sm.tile([P, NP], F32, tag=f"e{img}")
    nc.scalar.activation(e, dsel[:, :, 2], mybir.ActivationFunctionType.Exp)
    nc.vector.tensor_mul(h, h, e)
    nc.scalar.activation(e, dsel[:, :, 3], mybir.ActivationFunctionType.Exp)
    nc.vector.tensor_mul(w, w, e)

    ref = sm.tile([P, NP, 4], F32, tag=f"ref{img}")
    nc.vector.tensor_scalar_mul(t1, h, 0.5)
    nc.vector.tensor_sub(ref[:, :, 0], cy, t1)
    nc.vector.tensor_add(ref[:, :, 2], cy, t1)
    nc.vector.tensor_scalar_mul(t2, w, 0.5)
    nc.vector.tensor_sub(ref[:, :, 1], cx, t2)
    nc.vector.tensor_add(ref[:, :, 3], cx, t2)
    nc.vector.tensor_scalar(
        out=ref, in0=ref, scalar1=0.0, scalar2=1.0,
        op0=mybir.AluOpType.max, op1=mybir.AluOpType.min,
    )

    # ---- NMS state ----
    sc = state["sc"][:, img]
    ob = state["ob"][:, img]
    ar = state["ar"][:, img]
    cat = state["cat"][:, img]
    negs = state["negs"]

    vf = sm.tile([P, NP], F32, tag=f"vf{img}")
    nc.vector.tensor_single_scalar(vf, cid, 0.5, op=mybir.AluOpType.is_ge)
    v = sm.tile([P, NP], mybir.dt.uint8, tag=f"v{img}")
    nc.vector.tensor_mul(v, vf, ge)
    nc.vector.tensor_copy(sc, negs)
    nc.vector.copy_predicated(sc, v, scores)

    nc.vector.scalar_tensor_tensor(
        out=ob, in0=cid.unsqueeze(2).to_broadcast([P, NP, 4]), scalar=2.0,
        in1=ref, op0=mybir.AluOpType.mult, op1=mybir.AluOpType.add,
    )
    ar2 = sm.tile([P, NP, 2], F32, tag=f"ar2{img}")
    nc.vector.tensor_sub(ar2, ob[:, :, 2:4], ob[:, :, 0:2])
    nc.vector.tensor_mul(ar, ar2[:, :, 0], ar2[:, :, 1])
    nc.vector.tensor_copy(cat[:, :, 0:4], ref)
    nc.vector.tensor_copy(cat[:, :, 4], cid)
    nc.vector.tensor_copy(cat[:, :, 5], scores)


def _nms_image(nc, tc, sm, img, det, state):
    """Cold path per image: fixed K-iteration NMS; rows past exhaustion are
    written as exact zeros (gm == NEG gate)."""
    sc = state["sc"][:, img]
    ob = state["ob"][:, img]
    ar = state["ar"][:, img]
    cat = state["cat"][:, img]
    negs = state["negs"]
    mr = state["mr"]

    with tc.For_i(0, K, name=f"nms{img}") as i:
        pm = sm.tile([P, 1], F32, tag=f"pm{img}")
        nc.vector.reduce_max(pm, sc, axis=mybir.AxisListType.X)
        gm = sm.tile([P, 1], F32, tag=f"gm{img}")
        nc.gpsimd.partition_all_reduce(gm, pm, channels=P,
                                       reduce_op=bass_isa.ReduceOp.max)
        msk = sm.tile([P, NP], F32, tag=f"msk{img}")
        nc.vector.tensor_tensor(msk, sc, gm.to_broadcast([P, NP]),
                                op=mybir.AluOpType.is_equal)
        mb6 = sm.tile([P, NP, 6], F32, tag=f"mb6{img}")
        nc.vector.tensor_tensor(
            mb6, cat, msk.unsqueeze(2).to_broadcast([P, NP, 6]),
            op=mybir.AluOpType.mult,
        )
        r6p = sm.tile([P, 6], F32, tag=f"r6p{img}")
        nc.vector.reduce_sum(r6p, mb6.rearrange("p n k -> p k n"),
                             axis=mybir.AxisListType.X)
        r6 = sm.tile([P, 6], F32, tag=f"r6{img}")
        nc.gpsimd.partition_all_reduce(r6, r6p, channels=P,
                                       reduce_op=bass_isa.ReduceOp.add)
        okm = sm.tile([P, 1], F32, tag=f"okm{img}")
        nc.vector.tensor_single_scalar(okm, gm, NEG * 0.5,
                                       op=mybir.AluOpType.is_gt)
        nc.vector.tensor_mul(r6, r6, okm.to_broadcast([P, 6]))
        nc.vector.tensor_copy(det[img][0:1, bass.ds(i * 6, 6)],
                              r6[0:1, :])

        sb = sm.tile([P, 4], F32, tag=f"sb{img}")
        nc.vector.scalar_tensor_tensor(
            out=sb, in0=r6[:, 4:5].to_broadcast([P, 4]), scalar=2.0,
            in1=r6[:, 0:4], op0=mybir.AluOpType.mult, op1=mybir.AluOpType.add,
        )
        mx = sm.tile([P, NP, 2], F32, tag=f"mx{img}")
        nc.vector.tensor_tensor(
            mx, ob[:, :, 0:2], sb[:, 0:2].unsqueeze(1).to_broadcast([P, NP, 2]),
            op=mybir.AluOpType.max,
        )
        mn = sm.tile([P, NP, 2], F32, tag=f"mn{img}")
        nc.vector.tensor_tensor(
            mn, ob[:, :, 2:4], sb[:, 2:4].unsqueeze(1).to_broadcast([P, NP, 2]),
            op=mybir.AluOpType.min,
        )
        nc.vector.tensor_sub(mn, mn, mx)
        nc.vector.tensor_scalar_max(mn, mn, 0.0)
        inter = sm.tile([P, NP], F32, tag=f"inter{img}")
        nc.vector.tensor_mul(inter, mn[:, :, 0], mn[:, :, 1])
        aa2 = sm.tile([P, 2], F32, tag=f"aa2{img}")
        nc.vector.tensor_sub(aa2, sb[:, 2:4], sb[:, 0:2])
        aa = sm.tile([P, 1], F32, tag=f"aa{img}")
        nc.vector.tensor_mul(aa, aa2[:, 0:1], aa2[:, 1:2])
        u = sm.tile([P, NP], F32, tag=f"u{img}")
        nc.vector.scalar_tensor_tensor(
            out=u, in0=ar, scalar=aa[:, 0:1], in1=inter,
            op0=mybir.AluOpType.add, op1=mybir.AluOpType.subtract,
        )
        sup = sm.tile([P, NP], mybir.dt.uint8, tag=f"sup{img}")
        nc.vector.scalar_tensor_tensor(
            out=sup, in0=u, scalar=NMS_T, in1=inter,
            op0=mybir.AluOpType.mult, op1=mybir.AluOpType.is_lt,
        )
        nc.vector.copy_predicated(sc, sup, negs)
        nc.vector.tensor_copy(mr[:, 0:1], gm)
        nc.vector.match_replace(out=sc, in_to_replace=mr, in_values=sc,
                                imm_value=NEG)


def build_nc():
    nc = bacc.Bacc("TRN2", target_bir_lowering=False)
    rois_t = nc.dram_tensor("rois", [BPC, N, 4], F32, kind="ExternalInput")
    probs_t = nc.dram_tensor("probs", [BPC, N, C], F32, kind="ExternalInput")
    deltas_t = nc.dram_tensor("deltas", [BPC, N, C, 4], F32, kind="ExternalInput")
    out_t = nc.dram_tensor("out", [BPC, K, 6], F32, kind="ExternalOutput")
    dbg_t = None
    if DEBUG:
        dbg_t = nc.dram_tensor("dbg", [1, 16], F32, kind="ExternalOutput")

    with TileContext(nc) as tc:
        with (
            tc.tile_pool(name="big", bufs=1) as big,
            tc.tile_pool(name="small", bufs=1) as sm,
            tc.tile_pool(name="psum", bufs=1, space="PSUM") as pp,
        ):
            # ---------------- fast path ----------------
            # probs for both images in one twin tile; one DMA per image,
            # issued on separate HWDGE queues (SP + Act) so descriptor
            # generation for the two transfers runs in parallel
            ptw = big.tile([P, BPC, NP, C], F32, tag="probs")
            p1 = probs_t[1].rearrange("(p n) c -> p n c", p=P)
            nc.sync.dma_start(
                out=ptw[:, 0],
                in_=probs_t[0].rearrange("(p n) c -> p n c", p=P))
            nc.sync.dma_start(out=ptw[:, 1, 0:BS], in_=p1[:, 0:BS])
            nc.scalar.dma_start(out=ptw[:, 1, BS:NP], in_=p1[:, BS:NP])

            det0 = sm.tile([1, K * 6], F32, tag="det0")
            det1 = sm.tile([1, K * 6], F32, tag="det1")
            det = [det0, det1]
            nc.vector.memset(det0, 0.0)
            nc.gpsimd.memset(det1, 0.0)

            # zeros out-DMA up front; real detections overwrite in the guard
            out_aps = []
            for img in range(BPC):
                ap = out_t[img].rearrange("k s -> (k s)").unsqueeze(0)
                out_aps.append(ap)
                nc.sync.dma_start(out=ap, in_=det[img][0:1])

            # element count >= MIN_CONF: DVE is_ge+sum chunks (coeff 2)
            # then ACT sign+sum chunks (coeff 1); cnt cols = DVE then ACT
            NCOL = 2 + sum(1 for n in (A_ACT, B_ACT, C_ACT) if n > 0)
            cnt = sm.tile([P, NCOL], F32, tag="cnt")
            scrA = sm.tile([P, max(A_DVE, B_DVE), C], mybir.dt.uint8,
                           tag="scrA")
            scrB = sm.tile([P, max(A_ACT, B_ACT, C_ACT), C],
                           mybir.dt.bfloat16, tag="scrB")
            biasT = sm.tile([P, 1], F32, tag="biasT")
            nc.gpsimd.memset(biasT, -MIN_CONF)

            def dve_count(src_ap, col, nr):
                nc.vector.tensor_scalar(
                    out=scrA[:, 0:nr], in0=src_ap, scalar1=MIN_CONF,
                    scalar2=None, op0=mybir.AluOpType.is_ge,
                    op1=mybir.AluOpType.add,
                    accum_out=cnt[:, col:col + 1],
                )

            def act_count(src_ap, col, nr):
                nc.scalar.activation(
                    scrB[:, 0:nr], src_ap,
                    mybir.ActivationFunctionType.Sign, bias=biasT[:, 0:1],
                    accum_out=cnt[:, col:col + 1],
                )

            # DVE: img0 head chunk, then img1 head chunk (arrival order)
            dve_count(ptw[:, 0, 0:A_DVE], 0, A_DVE)
            dve_count(ptw[:, 1, 0:B_DVE], 1, B_DVE)
            # ACT: img0 tail, img1 mid (sync), img1 tail (act queue, last)
            col = 2
            if A_ACT > 0:
                act_count(ptw[:, 0, A_DVE:NP], col, A_ACT)
                col += 1
            if B_ACT > 0:
                act_count(ptw[:, 1, B_DVE:BS], col, B_ACT)
                col += 1
            if C_ACT > 0:
                act_count(ptw[:, 1, BS:NP], col, C_ACT)

            ones = sm.tile([P, 1], F32, tag="ones")
            nc.vector.memset(ones, 1.0)
            # g = 2*sum(DVE counts) + sum(ACT sign sums) + #ACT-elems
            #   = 2 * (total elements >= MIN_CONF)   (exact in f32)
            nc.vector.tensor_scalar_mul(cnt[:, 0:2], cnt[:, 0:2], 2.0)
            csum = pp.tile([1, NCOL], F32, tag="csum")
            nc.tensor.matmul(csum, ones, cnt, start=True, stop=True)
            ga = sm.tile([1, 1], F32, tag="ga")
            nc.vector.reduce_sum(ga, csum, axis=mybir.AxisListType.X)
            gi = sm.tile([1, 1], I32, tag="gi")
            nc.vector.tensor_scalar(
                out=gi, in0=ga, scalar1=TOTAL_B_ELEMS, scalar2=None,
                op0=mybir.AluOpType.add, op1=mybir.AluOpType.bypass,
            )

            if DEBUG:
                dbgs = sm.tile([1, 8], F32, tag="dbgs")
                nc.vector.memset(dbgs, 0.0)
                nc.vector.tensor_copy(dbgs[0:1, 0:4], cs)
                nc.vector.tensor_copy(dbgs[0:1, 4:5], ga)
                nc.vector.tensor_copy(dbgs[0:1, 5:6], gb)
                nc.sync.dma_start(out=dbg_t[0:1, 0:8], in_=dbgs)

            gv = nc.values_load(gi[0:1, 0:1], min_val=0,
                                max_val=2 * BPC * N * C,
                                skip_runtime_bounds_check=True)

            # ---------------- guarded cold path ----------------
            if not NOGUARD:
              with tc.If(gv >= 1):
                crev = sm.tile([P, NP, C], F32, tag="crev")
                nc.gpsimd.iota(crev, pattern=[[0, NP], [-1, C]], base=C - 1,
                               channel_multiplier=0,
                               allow_small_or_imprecise_dtypes=True)
                negs = sm.tile([P, NP], F32, tag="negs")
                nc.gpsimd.memset(negs, NEG)
                mr = sm.tile([P, 8], F32, tag="mr")
                nc.gpsimd.memset(mr, NEG)

                sc_w = sm.tile([P, BPC, NP], F32, tag="sc")
                ob_w = sm.tile([P, BPC, NP, 4], F32, tag="ob")
                ar_w = sm.tile([P, BPC, NP], F32, tag="ar")
                cat_w = sm.tile([P, BPC, NP, 6], F32, tag="cat")
                state = {
                    "negs": negs,
                    "mr": mr,
                    "sc": sc_w,
                    "ob": ob_w,
                    "ar": ar_w,
                    "cat": cat_w,
                }
                scw = sm.tile([P, BPC, NP], F32, tag="scores")

                for img in range(BPC):
                    rt = sm.tile([P, NP, 4], F32, tag=f"rois{img}")
                    nc.sync.dma_start(
                        out=rt,
                        in_=rois_t[img].rearrange("(p n) k -> p n k", p=P))
                    dt_ = big.tile([P, NP, C, 4], F32, tag=f"deltas{img}")
                    dsrc = deltas_t[img].rearrange("(p n) c k -> p n c k", p=P)
                    for s in range(8):
                        sl = slice(16 * s, 16 * s + 16)
                        nc.sync.dma_start(out=dt_[sl], in_=dsrc[sl])
                    _refine_image(nc, tc, sm, img, ptw, scw, rt, dt_, crev,
                                  state)
                if DEBUG:
                    pmd = sm.tile([P, 4], F32, tag="pmd")
                    nc.vector.reduce_max(pmd[:, 0:1], state["sc"][:, 0],
                                         axis=mybir.AxisListType.X)
                    nc.vector.reduce_max(pmd[:, 1:2], state["sc"][:, 1],
                                         axis=mybir.AxisListType.X)
                    nc.vector.reduce_max(pmd[:, 2:3], scw[:, 0],
                                         axis=mybir.AxisListType.X)
                    nc.vector.reduce_max(pmd[:, 3:4], scw[:, 1],
                                         axis=mybir.AxisListType.X)
                    pmg = sm.tile([P, 4], F32, tag="pmg")
                    nc.gpsimd.partition_all_reduce(
                        pmg, pmd, channels=P, reduce_op=bass_isa.ReduceOp.max)
                    nc.sync.dma_start(out=dbg_t[0:1, 8:12], in_=pmg[0:1, :])
                for img in range(BPC):
                    _nms_image(nc, tc, sm, img, det, state)
                if DEBUG:
                    dbgs2 = sm.tile([1, 4], F32, tag="dbgs2")
                    nc.vector.tensor_copy(dbgs2[0:1, 0:2], det[0][0:1, 0:2])
                    nc.vector.tensor_copy(dbgs2[0:1, 2:4], det[1][0:1, 0:2])
                    nc.sync.dma_start(out=dbg_t[0:1, 12:16], in_=dbgs2)
                for img in range(BPC):
                    fap = out_t[img].rearrange("k s -> (k s)").unsqueeze(0)
                    nc.sync.dma_start(out=fap, in_=det[img][0:1])
    nc.compile()
    return nc


LAST_RESULTS = None  # BassKernelResults of the most recent kernel() call


def kernel(rois, probs, deltas):
    global LAST_RESULTS
    from concourse import bass_utils

    nc = build_nc()
    in_maps = []
    for c in range(NCORES):
        sl = slice(c * BPC, (c + 1) * BPC)
        in_maps.append({
            "rois": np.ascontiguousarray(rois[sl], dtype=np.float32),
            "probs": np.ascontiguousarray(probs[sl], dtype=np.float32),
            "deltas": np.ascontiguousarray(deltas[sl], dtype=np.float32),
        })
    res = bass_utils.run_bass_kernel_spmd(nc, in_maps, core_ids=list(range(NCORES)))
    LAST_RESULTS = res
    return np.concatenate([r["out"] for r in res.results], axis=0)


if __name__ == "__main__":
    rng = np.random.default_rng(0)
    out = kernel(
        rng.random((B, N, 4), np.float32),
        rng.random((B, N, C), np.float32),
        rng.standard_normal((B, N, C, 4)).astype(np.float32),
    )
    print(out.shape, np.abs(out).max())


# revision 20
# speedup vs baseline: 1.0309x; 1.0154x over previous
"""Trainium2 Bass kernel for DetectionLayer (refine + per-class NMS).

Contract: kernel(rois, probs, deltas) with FULL inputs
  rois   [16, 4096, 4]   f32
  probs  [16, 4096, 81]  f32
  deltas [16, 4096, 81, 4] f32
returns [16, 100, 6] f32 detections, matching the jax reference.

Sharding: pure data parallel - 2 images per core across 8 NeuronCores.

Fast path (always): DMA both images' probs, count elements >= 0.7 with a
DVE is_ge+accum / ACT sign+accum split, sum via PE ones-matmul.  The
zeroed output is DMA'd to HBM up front.
Guard (tc.If, only when count > 0): deltas load, per-argmax-class box
refine, and a fixed 100-iteration per-class NMS per image, then the real
detections overwrite the zeros in HBM.
"""

import os as _os

import numpy as np

import concourse.bacc as bacc
import concourse.bass as bass
import concourse.bass_isa as bass_isa
import concourse.mybir as mybir
from concourse.tile import TileContext

B = 16              # full batch
NCORES = 8
BPC = B // NCORES   # images per core
N = 4096            # rois per image
C = 81              # classes
K = 100             # detection_max_instances
P = 128             # SBUF partitions
NP = N // P         # rois per partition per image (32)
NEG = -1e9
MIN_CONF = 0.7
NMS_T = 0.3
F32 = mybir.dt.float32
I32 = mybir.dt.int32

# gate split: DVE handles rois [0, NA), ACT handles [NA, NP) of each image
DEBUG = _os.environ.get("DETK_DEBUG", "0") == "1"
NOGUARD = _os.environ.get("DETK_NOGUARD", "0") == "1"
# probs DMA chunks in issue order: (img, lo, hi, dve_rois, queue)
# queue 0 = SP HWDGE (rings prioritize it), 1 = Act HWDGE (lands last).
# Per chunk, DVE counts rois [lo, lo+dve) via is_ge+sum (coeff 2) and the
# ACT engine does [lo+dve, hi) via sign+sum (coeff 1). Chunks are sized so
# the last-landing chunks carry little gate work.
CHUNKS = [
    (0, 0, 32, 18, 0),
    (1, 0, 18, 13, 0),
    (1, 18, 26, 4, 0),
    (1, 26, 32, 0, 1),
]
_DVE_N = [d for (_, _, _, d, _) in CHUNKS if d > 0]
_ACT_N = [hi - lo - d for (_, lo, hi, d, _) in CHUNKS if hi - lo - d > 0]
# total elements processed by ACT (sign-sum) chunks
TOTAL_B_ELEMS = float(P * C * sum(_ACT_N))


def _refine_image(nc, tc, sm, img, ptw, scw, rt, dt_, crev, state):
    """Cold path per image: select argmax-class delta, refine boxes, build
    NMS state. All tiles are [..] slices of twin tensors at free index img."""
    pt = ptw[:, img]          # [P, NP, C]
    scores = scw[:, img]      # [P, NP]

    nc.vector.reduce_max(scores, pt, axis=mybir.AxisListType.X)
    ge = sm.tile([P, NP], F32, tag=f"ge{img}")
    nc.vector.tensor_single_scalar(ge, scores, MIN_CONF,
                                   op=mybir.AluOpType.is_ge)

    # one-hot mask of argmax class: M = (probs == score), in place over probs
    m = pt
    nc.vector.tensor_tensor(
        m, pt, scores.unsqueeze(2).to_broadcast([P, NP, C]),
        op=mybir.AluOpType.is_equal,
    )

    # select argmax-class delta: deltas *= M (bcast over k), sum over c
    d_perm = dt_.rearrange("p n c k -> p n k c")
    nc.vector.tensor_tensor(
        d_perm, d_perm, m.unsqueeze(2).to_broadcast([P, NP, 4, C]),
        op=mybir.AluOpType.mult,
    )
    dsel = sm.tile([P, NP, 4], F32, tag=f"dsel{img}")
    nc.vector.reduce_sum(dsel, d_perm, axis=mybir.AxisListType.X)

    # class id = 80 - max((80-c) * M)  (ties -> smallest c, like argmax)
    nc.vector.tensor_tensor(m, m, crev, op=mybir.AluOpType.mult)
    cid = sm.tile([P, NP], F32, tag=f"cid{img}")
    nc.vector.reduce_max(cid, m, axis=mybir.AxisListType.X)
    nc.vector.tensor_scalar(
        out=cid, in0=cid, scalar1=-1.0, scalar2=float(C - 1),
        op0=mybir.AluOpType.mult, op1=mybir.AluOpType.add,
    )

    # bbox_std scaling (match reference op order exactly)
    nc.vector.tensor_scalar_mul(dsel[:, :, 0:2], dsel[:, :, 0:2], 0.1)
    nc.vector.tensor_scalar_mul(dsel[:, :, 2:4], dsel[:, :, 2:4], 0.2)

    # ---- apply deltas + clip (mirrors _apply_deltas fp32 op order) ----
    h = sm.tile([P, NP], F32, tag=f"h{img}")
    w = sm.tile([P, NP], F32, tag=f"w{img}")
    nc.vector.tensor_sub(h, rt[:, :, 2], rt[:, :, 0])
    nc.vector.tensor_sub(w, rt[:, :, 3], rt[:, :, 1])
    t1 = sm.tile([P, NP], F32, tag=f"t1{img}")
    t2 = sm.tile([P, NP], F32, tag=f"t2{img}")
    cy = sm.tile([P, NP], F32, tag=f"cy{img}")
    cx = sm.tile([P, NP], F32, tag=f"cx{img}")
    nc.vector.tensor_scalar_mul(t1, h, 0.5)
    nc.vector.tensor_add(t2, rt[:, :, 0], t1)
    nc.vector.tensor_mul(t1, dsel[:, :, 0], h)
    nc.vector.tensor_add(cy, t2, t1)
    nc.vector.tensor_scalar_mul(t1, w, 0.5)
    nc.vector.tensor_add(t2, rt[:, :, 1], t1)
    nc.vector.tensor_mul(t1, dsel[:, :, 1], w)
    nc.vector.tensor_add(cx, t2, t1)
    e = sm.tile([P, NP], F32, tag=f"e{img}")
    nc.scalar.activation(e, dsel[:, :, 2], mybir.ActivationFunctionType.Exp)
    nc.vector.tensor_mul(h, h, e)
    nc.scalar.activation(e, dsel[:, :, 3], mybir.ActivationFunctionType.Exp)
    nc.vector.tensor_mul(w, w, e)

    ref = sm.tile([P, NP, 4], F32, tag=f"ref{img}")
    nc.vector.tensor_scalar_mul(t1, h, 0.5)
    nc.vector.tensor_sub(ref[:, :, 0], cy, t1)
    nc.vector.tensor_add(ref[:, :, 2], cy, t1)
    nc.vector.tensor_scalar_mul(t2, w, 0.5)
    nc.vector.tensor_sub(ref[:, :, 1], cx, t2)
    nc.vector.tensor_add(ref[:, :, 3], cx, t2)
    nc.vector.tensor_scalar(
        out=ref, in0=ref, scalar1=0.0, scalar2=1.0,
        op0=mybir.AluOpType.max, op1=mybir.AluOpType.min,
    )

    # ---- NMS state ----
    sc = state["sc"][:, img]
    ob = state["ob"][:, img]
    ar = state["ar"][:, img]
    cat = state["cat"][:, img]
    negs = state["negs"]

    vf = sm.tile([P, NP], F32, tag=f"vf{img}")
    nc.vector.tensor_single_scalar(vf, cid, 0.5, op=mybir.AluOpType.is_ge)
    v = sm.tile([P, NP], mybir.dt.uint8, tag=f"v{img}")
    nc.vector.tensor_mul(v, vf, ge)
    nc.vector.tensor_copy(sc, negs)
    nc.vector.copy_predicated(sc, v, scores)

    nc.vector.scalar_tensor_tensor(
        out=ob, in0=cid.unsqueeze(2).to_broadcast([P, NP, 4]), scalar=2.0,
        in1=ref, op0=mybir.AluOpType.mult, op1=mybir.AluOpType.add,
    )
    ar2 = sm.tile([P, NP, 2], F32, tag=f"ar2{img}")
    nc.vector.tensor_sub(ar2, ob[:, :, 2:4], ob[:, :, 0:2])
    nc.vector.tensor_mul(ar, ar2[:, :, 0], ar2[:, :, 1])
    nc.vector.tensor_copy(cat[:, :, 0:4], ref)
    nc.vector.tensor_copy(cat[:, :, 4], cid)
    nc.vector.tensor_copy(cat[:, :, 5], scores)


def _nms_image(nc, tc, sm, img, det, state):
    """Cold path per image: fixed K-iteration NMS; rows past exhaustion are
    written as exact zeros (gm == NEG gate)."""
    sc = state["sc"][:, img]
    ob = state["ob"][:, img]
    ar = state["ar"][:, img]
    cat = state["cat"][:, img]
    negs = state["negs"]
    mr = state["mr"]

    with tc.For_i(0, K, name=f"nms{img}") as i:
        pm = sm.tile([P, 1], F32, tag=f"pm{img}")
        nc.vector.reduce_max(pm, sc, axis=mybir.AxisListType.X)
        gm = sm.tile([P, 1], F32, tag=f"gm{img}")
        nc.gpsimd.partition_all_reduce(gm, pm, channels=P,
                                       reduce_op=bass_isa.ReduceOp.max)
        msk = sm.tile([P, NP], F32, tag=f"msk{img}")
        nc.vector.tensor_tensor(msk, sc, gm.to_broadcast([P, NP]),
                                op=mybir.AluOpType.is_equal)
        mb6 = sm.tile([P, NP, 6], F32, tag=f"mb6{img}")
        nc.vector.tensor_tensor(
            mb6, cat, msk.unsqueeze(2).to_broadcast([P, NP, 6]),
            op=mybir.AluOpType.mult,
        )
        r6p = sm.tile([P, 6], F32, tag=f"r6p{img}")
        nc.vector.reduce_sum(r6p, mb6.rearrange("p n k -> p k n"),
                             axis=mybir.AxisListType.X)
        r6 = sm.tile([P, 6], F32, tag=f"r6{img}")
        nc.gpsimd.partition_all_reduce(r6, r6p, channels=P,
                                       reduce_op=bass_isa.ReduceOp.add)
        okm = sm.tile([P, 1], F32, tag=f"okm{img}")
        nc.vector.tensor_single_scalar(okm, gm, NEG * 0.5,
                                       op=mybir.AluOpType.is_gt)
        nc.vector.tensor_mul(r6, r6, okm.to_broadcast([P, 6]))
        nc.vector.tensor_copy(det[img][0:1, bass.ds(i * 6, 6)],
                              r6[0:1, :])

        sb = sm.tile([P, 4], F32, tag=f"sb{img}")
        nc.vector.scalar_tensor_tensor(
            out=sb, in0=r6[:, 4:5].to_broadcast([P, 4]), scalar=2.0,
            in1=r6[:, 0:4], op0=mybir.AluOpType.mult, op1=mybir.AluOpType.add,
        )
        mx = sm.tile([P, NP, 2], F32, tag=f"mx{img}")
        nc.vector.tensor_tensor(
            mx, ob[:, :, 0:2], sb[:, 0:2].unsqueeze(1).to_broadcast([P, NP, 2]),
            op=mybir.AluOpType.max,
        )
        mn = sm.tile([P, NP, 2], F32, tag=f"mn{img}")
        nc.vector.tensor_tensor(
            mn, ob[:, :, 2:4], sb[:, 2:4].unsqueeze(1).to_broadcast([P, NP, 2]),
            op=mybir.AluOpType.min,
        )
        nc.vector.tensor_sub(mn, mn, mx)
        nc.vector.tensor_scalar_max(mn, mn, 0.0)
        inter = sm.tile([P, NP], F32, tag=f"inter{img}")
        nc.vector.tensor_mul(inter, mn[:, :, 0], mn[:, :, 1])
        aa2 = sm.tile([P, 2], F32, tag=f"aa2{img}")
        nc.vector.tensor_sub(aa2, sb[:, 2:4], sb[:, 0:2])
        aa = sm.tile([P, 1], F32, tag=f"aa{img}")
        nc.vector.tensor_mul(aa, aa2[:, 0:1], aa2[:, 1:2])
        u = sm.tile([P, NP], F32, tag=f"u{img}")
        nc.vector.scalar_tensor_tensor(
            out=u, in0=ar, scalar=aa[:, 0:1], in1=inter,
            op0=mybir.AluOpType.add, op1=mybir.AluOpType.subtract,
        )
        sup = sm.tile([P, NP], mybir.dt.uint8, tag=f"sup{img}")
        nc.vector.scalar_tensor_tensor(
            out=sup, in0=u, scalar=NMS_T, in1=inter,
            op0=mybir.AluOpType.mult, op1=mybir.AluOpType.is_lt,
        )
        nc.vector.copy_predicated(sc, sup, negs)
        nc.vector.tensor_copy(mr[:, 0:1], gm)
        nc.vector.match_replace(out=sc, in_to_replace=mr, in_values=sc,
                                imm_value=NEG)


def build_nc():
    nc = bacc.Bacc("TRN2", target_bir_lowering=False)
    rois_t = nc.dram_tensor("rois", [BPC, N, 4], F32, kind="ExternalInput")
    probs_t = nc.dram_tensor("probs", [BPC, N, C], F32, kind="ExternalInput")
    deltas_t = nc.dram_tensor("deltas", [BPC, N, C, 4], F32, kind="ExternalInput")
    out_t = nc.dram_tensor("out", [BPC, K, 6], F32, kind="ExternalOutput")
    dbg_t = None
    if DEBUG:
        dbg_t = nc.dram_tensor("dbg", [1, 16], F32, kind="ExternalOutput")

    with TileContext(nc) as tc:
        with (
            tc.tile_pool(name="big", bufs=1) as big,
            tc.tile_pool(name="small", bufs=1) as sm,
            tc.tile_pool(name="psum", bufs=1, space="PSUM") as pp,
        ):
            # ---------------- fast path ----------------
            # probs for both images in one twin tile, DMA'd per CHUNKS on
            # two HWDGE queues (SP + Act) for parallel descriptor gen
            ptw = big.tile([P, BPC, NP, C], F32, tag="probs")
            psrc = [probs_t[b].rearrange("(p n) c -> p n c", p=P)
                    for b in range(BPC)]
            for b, lo, hi, _, q in CHUNKS:
                eng = nc.sync if q == 0 else nc.scalar
                eng.dma_start(out=ptw[:, b, lo:hi], in_=psrc[b][:, lo:hi])

            det0 = sm.tile([1, K * 6], F32, tag="det0")
            det1 = sm.tile([1, K * 6], F32, tag="det1")
            det = [det0, det1]
            nc.vector.memset(det0, 0.0)
            nc.gpsimd.memset(det1, 0.0)

            # zeros out-DMA up front; real detections overwrite in the guard
            out_aps = []
            for img in range(BPC):
                ap = out_t[img].rearrange("k s -> (k s)").unsqueeze(0)
                out_aps.append(ap)
                nc.sync.dma_start(out=ap, in_=det[img][0:1])

            # element count >= MIN_CONF: DVE is_ge+sum (coeff 2) then
            # ACT sign+sum (coeff 1); cnt cols = DVE chunks then ACT chunks
            NDVE = len(_DVE_N)
            NCOL = NDVE + len(_ACT_N)
            cnt = sm.tile([P, NCOL], F32, tag="cnt")
            scrA = sm.tile([P, max(_DVE_N), C], mybir.dt.uint8, tag="scrA")
            scrB = sm.tile([P, max(_ACT_N), C], mybir.dt.bfloat16,
                           tag="scrB")
            biasT = sm.tile([P, 1], F32, tag="biasT")
            nc.gpsimd.memset(biasT, -MIN_CONF)

            col_dve, col_act = 0, NDVE
            for b, lo, hi, dve_n, _ in CHUNKS:
                if dve_n > 0:
                    nc.vector.tensor_scalar(
                        out=scrA[:, 0:dve_n], in0=ptw[:, b, lo:lo + dve_n],
                        scalar1=MIN_CONF, scalar2=None,
                        op0=mybir.AluOpType.is_ge, op1=mybir.AluOpType.add,
                        accum_out=cnt[:, col_dve:col_dve + 1],
                    )
                    col_dve += 1
                act_n = hi - lo - dve_n
                if act_n > 0:
                    nc.scalar.activation(
                        scrB[:, 0:act_n], ptw[:, b, lo + dve_n:hi],
                        mybir.ActivationFunctionType.Sign,
                        bias=biasT[:, 0:1],
                        accum_out=cnt[:, col_act:col_act + 1],
                    )
                    col_act += 1

            ones = sm.tile([P, 1], F32, tag="ones")
            nc.vector.memset(ones, 1.0)
            # g = 2*sum(DVE counts) + sum(ACT sign sums) + #ACT-elems
            #   = 2 * (total elements >= MIN_CONF)   (exact in f32)
            nc.vector.tensor_scalar_mul(cnt[:, 0:NDVE], cnt[:, 0:NDVE], 2.0)
            csum = pp.tile([1, NCOL], F32, tag="csum")
            nc.tensor.matmul(csum, ones, cnt, start=True, stop=True)
            ga = sm.tile([1, 1], F32, tag="ga")
            nc.vector.reduce_sum(ga, csum, axis=mybir.AxisListType.X)
            gi = sm.tile([1, 1], I32, tag="gi")
            nc.vector.tensor_scalar(
                out=gi, in0=ga, scalar1=TOTAL_B_ELEMS, scalar2=None,
                op0=mybir.AluOpType.add, op1=mybir.AluOpType.bypass,
            )

            if DEBUG:
                dbgs = sm.tile([1, 8], F32, tag="dbgs")
                nc.vector.memset(dbgs, 0.0)
                nc.vector.tensor_copy(dbgs[0:1, 0:4], cs)
                nc.vector.tensor_copy(dbgs[0:1, 4:5], ga)
                nc.vector.tensor_copy(dbgs[0:1, 5:6], gb)
                nc.sync.dma_start(out=dbg_t[0:1, 0:8], in_=dbgs)

            gv = nc.values_load(gi[0:1, 0:1], min_val=0,
                                max_val=2 * BPC * N * C,
                                skip_runtime_bounds_check=True)

            # ---------------- guarded cold path ----------------
            if not NOGUARD:
              with tc.If(gv >= 1):
                crev = sm.tile([P, NP, C], F32, tag="crev")
                nc.gpsimd.iota(crev, pattern=[[0, NP], [-1, C]], base=C - 1,
                               channel_multiplier=0,
                               allow_small_or_imprecise_dtypes=True)
                negs = sm.tile([P, NP], F32, tag="negs")
                nc.gpsimd.memset(negs, NEG)
                mr = sm.tile([P, 8], F32, tag="mr")
                nc.gpsimd.memset(mr, NEG)

                sc_w = sm.tile([P, BPC, NP], F32, tag="sc")
                ob_w = sm.tile([P, BPC, NP, 4], F32, tag="ob")
                ar_w = sm.tile([P, BPC, NP], F32, tag="ar")
                cat_w = sm.tile([P, BPC, NP, 6], F32, tag="cat")
                state = {
                    "negs": negs,
                    "mr": mr,
                    "sc": sc_w,
                    "ob": ob_w,
                    "ar": ar_w,
                    "cat": cat_w,
                }
                scw = sm.tile([P, BPC, NP], F32, tag="scores")

                for img in range(BPC):
                    rt = sm.tile([P, NP, 4], F32, tag=f"rois{img}")
                    nc.sync.dma_start(
                        out=rt,
                        in_=rois_t[img].rearrange("(p n) k -> p n k", p=P))
                    dt_ = big.tile([P, NP, C, 4], F32, tag=f"deltas{img}")
                    dsrc = deltas_t[img].rearrange("(p n) c k -> p n c k", p=P)
                    for s in range(8):
                        sl = slice(16 * s, 16 * s + 16)
                        nc.sync.dma_start(out=dt_[sl], in_=dsrc[sl])
                    _refine_image(nc, tc, sm, img, ptw, scw, rt, dt_, crev,
                                  state)
                if DEBUG:
                    pmd = sm.tile([P, 4], F32, tag="pmd")
                    nc.vector.reduce_max(pmd[:, 0:1], state["sc"][:, 0],
                                         axis=mybir.AxisListType.X)
                    nc.vector.reduce_max(pmd[:, 1:2], state["sc"][:, 1],
                                         axis=mybir.AxisListType.X)
                    nc.vector.reduce_max(pmd[:, 2:3], scw[:, 0],
                                         axis=mybir.AxisListType.X)
                    nc.vector.reduce_max(pmd[:, 3:4], scw[:, 1],
                                         axis=mybir.AxisListType.X)
                    pmg = sm.tile([P, 4], F32, tag="pmg")
                    nc.gpsimd.partition_all_reduce(
                        pmg, pmd, channels=P, reduce_op=bass_isa.ReduceOp.max)
                    nc.sync.dma_start(out=dbg_t[0:1, 8:12], in_=pmg[0:1, :])
                for img in range(BPC):
                    _nms_image(nc, tc, sm, img, det, state)
                if DEBUG:
                    dbgs2 = sm.tile([1, 4], F32, tag="dbgs2")
                    nc.vector.tensor_copy(dbgs2[0:1, 0:2], det[0][0:1, 0:2])
                    nc.vector.tensor_copy(dbgs2[0:1, 2:4], det[1][0:1, 0:2])
                    nc.sync.dma_start(out=dbg_t[0:1, 12:16], in_=dbgs2)
                for img in range(BPC):
                    fap = out_t[img].rearrange("k s -> (k s)").unsqueeze(0)
                    nc.sync.dma_start(out=fap, in_=det[img][0:1])
    nc.compile()
    return nc


LAST_RESULTS = None  # BassKernelResults of the most recent kernel() call


def kernel(rois, probs, deltas):
    global LAST_RESULTS
    from concourse import bass_utils

    nc = build_nc()
    in_maps = []
    for c in range(NCORES):
        sl = slice(c * BPC, (c + 1) * BPC)
        in_maps.append({
            "rois": np.ascontiguousarray(rois[sl], dtype=np.float32),
            "probs": np.ascontiguousarray(probs[sl], dtype=np.float32),
            "deltas": np.ascontiguousarray(deltas[sl], dtype=np.float32),
        })
    res = bass_utils.run_bass_kernel_spmd(nc, in_maps, core_ids=list(range(NCORES)))
    LAST_RESULTS = res
    return np.concatenate([r["out"] for r in res.results], axis=0)


if __name__ == "__main__":
    rng = np.random.default_rng(0)
    out = kernel(
        rng.random((B, N, 4), np.float32),
        rng.random((B, N, C), np.float32),
        rng.standard_normal((B, N, C, 4)).astype(np.float32),
    )
    print(out.shape, np.abs(out).max())
